# revision 33
# baseline (speedup 1.0000x reference)
"""PatchCore on 8 Trainium2 cores.

Pipeline per image (one image per core, memory bank replicated):
  - Host: materialize the reference's scrambled patch stream (pure gather, no
    arithmetic) in group-major layout; device computes all sums.
    feats[:, j<512]  = (sum of 9-group + center)/10 of the feat1 stream.
    feats[:, 512+j'] = bilinear14->28(18-group sums of the feat2 stream)/18.
  - kNN: d2 = ||q||^2 - 2 q.m + ||m||^2 over 20096 (padded) memory rows in bf16
    on the PE array, running min on DVE, -2s+mm fused on ACT.
  - sqrt + 28->224 bilinear resize as two interpolation matmuls.
"""

from contextlib import ExitStack

import ml_dtypes
import numpy as np
from numpy.lib.stride_tricks import sliding_window_view

import concourse.bacc as bacc
import concourse.bass as bass
import concourse.mybir as mybir
import concourse.tile as tile
from concourse.bass_utils import run_bass_kernel_spmd
from concourse.masks import make_identity

P = 128
NQ = 784          # 28*28 queries per image
QB = 392          # query block (2 blocks per psum-bank row budget)
NCHUNK = 157      # 20096 padded memory rows / 128
FP32 = mybir.dt.float32
BF16 = mybir.dt.bfloat16
AF = mybir.ActivationFunctionType
OP = mybir.AluOpType
AX = mybir.AxisListType


def _upsample2x(nc, pool, x, H, W, tag):
    """Bilinear align_corners=False 2x upsample of last axis: [128,H,W]->[128,H,2W]."""
    u = pool.tile([P, H, W], FP32, tag=f"{tag}_u", name=f"{tag}_u")
    nc.vector.tensor_scalar_mul(u[:], x[:], 0.25)
    v = pool.tile([P, H, W], FP32, tag=f"{tag}_v", name=f"{tag}_v")
    nc.vector.tensor_sub(v[:], x[:], u[:])
    o = pool.tile([P, H, 2 * W], FP32, tag=f"{tag}_o", name=f"{tag}_o")
    W2 = 2 * W
    # out[0] = x[0]; out[2W-1] = x[W-1]
    nc.vector.tensor_copy(o[:, :, 0 :: W2 - 1], x[:, :, 0 :: W - 1])
    # out[2t] = 0.25 x[t-1] + 0.75 x[t], t=1..W-1
    nc.vector.tensor_add(o[:, :, 2 : W2 - 1 : 2], u[:, :, 0 : W - 1], v[:, :, 1:W])
    # out[2t+1] = 0.75 x[t] + 0.25 x[t+1], t=0..W-2
    nc.vector.tensor_add(o[:, :, 1 : W2 - 2 : 2], v[:, :, 0 : W - 1], u[:, :, 1:W])
    return o


def _build_nc():
    nc = bacc.Bacc("TRN2", target_bir_lowering=False)

    s1_d = nc.dram_tensor("s1", [4, P, 9, NQ], BF16, kind="ExternalInput")
    s2_d = nc.dram_tensor("s2", [4, P, 18, 196], BF16, kind="ExternalInput")
    mt_d = nc.dram_tensor("memt", [NCHUNK, P, 8, P], BF16, kind="ExternalInput")
    mmn_d = nc.dram_tensor("mmn", [P, NCHUNK], FP32, kind="ExternalInput")
    rm_d = nc.dram_tensor("rmat", [28, 224], FP32, kind="ExternalInput")
    cm_d = nc.dram_tensor("cmat", [28, 224], FP32, kind="ExternalInput")
    raw_d = nc.dram_tensor("raw", [224, 224], FP32, kind="ExternalOutput")
    score_d = nc.dram_tensor("score", [1, 1], FP32, kind="ExternalOutput")

    with tile.TileContext(nc) as tc, ExitStack() as ctx:
        # ---- persistent tiles ----
        def _tile(shape, dtype, name):
            t, f = tc.tile(shape, dtype, name=name)
            ctx.callback(f)
            return t

        identity = _tile([P, P], FP32, name="identity")
        make_identity(nc, identity)
        ones = _tile([P, 1], FP32, name="ones")
        nc.vector.memset(ones[:], 1.0)
        runmin = _tile([P, NQ], FP32, name="runmin")
        nc.vector.memset(runmin[:], 3.0e38)
        sq_acc = _tile([P, NQ], FP32, name="sq_acc")
        mmn_sb = _tile([P, NCHUNK], FP32, name="mmn_sb")
        nc.sync.dma_start(mmn_sb[:], mmn_d[:])
        rm_sb = _tile([28, 224], FP32, name="rm_sb")
        nc.sync.dma_start(rm_sb[:], rm_d[:])
        cm_sb = _tile([28, 224], FP32, name="cm_sb")
        nc.sync.dma_start(cm_sb[:], cm_d[:])
        ps28T = _tile([28, 28], FP32, name="ps28T")
        e_bf = [_tile([P, NQ], BF16, name=f"ebf{i}") for i in range(8)]

        emb = tc.alloc_tile_pool(name="emb", bufs=2)

        # ---- E1: feats cols 0..511 = (9-group sum + center)/10 ----
        # tree-reduce keeps reader-instruction count per DMA'd tile low
        # (hardware DMA has a small sync-wait budget).
        for i in range(4):
            x = emb.tile([P, 9, NQ], BF16, tag="s1in", name="s1in")
            nc.sync.dma_start(x[:], s1_d[i])
            t4 = emb.tile([P, 4, NQ], FP32, tag="e1t4", name="e1t4")
            nc.vector.tensor_add(t4[:], x[:, 0:4, :], x[:, 4:8, :])
            c2 = emb.tile([P, NQ], FP32, tag="e1c2", name="e1c2")
            nc.vector.tensor_add(c2[:], x[:, 4, :], x[:, 8, :])
            t2 = emb.tile([P, 2, NQ], FP32, tag="e1t2", name="e1t2")
            nc.vector.tensor_add(t2[:], t4[:, 0:2, :], t4[:, 2:4, :])
            acc = emb.tile([P, NQ], FP32, tag="e1acc", name="e1acc")
            nc.vector.tensor_add(acc[:], t2[:, 0, :], t2[:, 1, :])
            nc.vector.tensor_add(acc[:], acc[:], c2[:])
            nc.scalar.activation(e_bf[i][:], acc[:], AF.Copy, scale=0.1)
            sqt = emb.tile([P, NQ], FP32, tag="sqt", name="sqt")
            nc.scalar.activation(sqt[:], e_bf[i][:], AF.Square)
            if i == 0:
                nc.vector.tensor_copy(sq_acc[:], sqt[:])
            else:
                nc.vector.tensor_add(sq_acc[:], sq_acc[:], sqt[:])

        # ---- E2: cols 512..1023 = bilinear14->28(18-group sums)/18 ----
        for i in range(4):
            y = emb.tile([P, 18, 196], BF16, tag="s2in", name="s2in")
            nc.sync.dma_start(y[:], s2_d[i])
            r9 = emb.tile([P, 9, 196], FP32, tag="e2r9", name="e2r9")
            nc.vector.tensor_add(r9[:], y[:, 0:9, :], y[:, 9:18, :])
            r4 = emb.tile([P, 4, 196], FP32, tag="e2r4", name="e2r4")
            nc.vector.tensor_add(r4[:], r9[:, 0:4, :], r9[:, 4:8, :])
            r2 = emb.tile([P, 2, 196], FP32, tag="e2r2", name="e2r2")
            nc.vector.tensor_add(r2[:], r4[:, 0:2, :], r4[:, 2:4, :])
            w2 = emb.tile([P, 14, 14], FP32, tag="w2", name="w2")
            w2f = w2.rearrange("p h w -> p (h w)")
            nc.vector.tensor_add(w2f, r2[:, 0, :], r2[:, 1, :])
            nc.vector.tensor_add(w2f, w2f, r9[:, 8, :])
            ow = _upsample2x(nc, emb, w2, 14, 14, "w")  # [128,14,28]
            ohin = ow.transpose([0, 2, 1])  # view [128,28,14]: expand h axis next
            oh = _upsample2x(nc, emb, ohin, 28, 14, "h")  # [128,28,28] as (w,h)
            # oh free layout is (w, h); rearrange view back to (h, w) row-major q
            nc.scalar.activation(
                e_bf[4 + i].rearrange("p (h w) -> p h w", h=28),
                oh.transpose([0, 2, 1]),
                AF.Copy,
                scale=1.0 / 18.0,
            )
            sqt = emb.tile([P, NQ], FP32, tag="sqt", name="sqt")
            nc.scalar.activation(sqt[:], e_bf[4 + i][:], AF.Square)
            nc.vector.tensor_add(sq_acc[:], sq_acc[:], sqt[:])
        emb.release()

        # ---- main kNN loop over 157 memory chunks ----
        mt_pool = ctx.enter_context(tc.tile_pool(name="mtp", bufs=6))
        tpool = ctx.enter_context(tc.tile_pool(name="tp", bufs=4))
        with tc.tile_pool(name="mmps", bufs=2, space=bass.MemorySpace.PSUM) as mmps:
            for c in range(NCHUNK):
                mt = mt_pool.tile([P, 8, P], BF16, tag="mt", name="mt")
                nc.sync.dma_start(mt[:], mt_d[c])
                for blk in range(2):
                    ps = mmps.tile([P, QB], FP32, tag=f"ps{blk}", name="ps")
                    for ki in range(8):
                        nc.tensor.matmul(
                            ps[:],
                            mt[:, ki, :],
                            e_bf[ki][:, blk * QB : (blk + 1) * QB],
                            start=(ki == 0),
                            stop=(ki == 7),
                        )
                    tt = tpool.tile([P, QB], FP32, tag=f"tt{blk}", name="tt")
                    nc.scalar.activation(
                        tt[:], ps[:], AF.Identity, bias=mmn_sb[:, c : c + 1], scale=-2.0
                    )
                    nc.vector.tensor_tensor(
                        runmin[:, blk * QB : (blk + 1) * QB],
                        runmin[:, blk * QB : (blk + 1) * QB],
                        tt[:],
                        op=OP.min,
                    )

        # ---- epilogue: per-query min over partitions, +qq, sqrt ----
        ep = ctx.enter_context(tc.tile_pool(name="ep", bufs=2))
        with tc.tile_pool(name="epps", bufs=1, space=bass.MemorySpace.PSUM) as epps:
            for t in range(28):
                tr = epps.tile([28, P], FP32, tag="tr", bufs=2, name="tr")
                nc.tensor.transpose(tr[:], runmin[:, 28 * t : 28 * (t + 1)], identity[:])
                qq_ps = epps.tile([28, 1], FP32, tag="qq", bufs=2, name="qq")
                nc.tensor.matmul(qq_ps[:], sq_acc[:, 28 * t : 28 * (t + 1)], ones[:])
                qq_sb = ep.tile([28, 1], FP32, tag="qqs", name="qqs")
                nc.scalar.copy(qq_sb[:], qq_ps[:])
                mn = ep.tile([28, 1], FP32, tag="mn", name="mn")
                nc.vector.tensor_reduce(mn[:], tr[:], axis=AX.X, op=OP.min)
                d2 = ep.tile([28, 1], FP32, tag="d2", name="d2")
                nc.vector.tensor_scalar(
                    d2[:], mn[:], qq_sb[:], 0.0, op0=OP.add, op1=OP.max
                )
                nc.scalar.activation(ps28T[:, t : t + 1], d2[:], AF.Sqrt)

            # ---- image score: max over all 784 ----
            colmax = ep.tile([28, 1], FP32, tag="cmx", name="cmx")
            nc.vector.tensor_reduce(colmax[:], ps28T[:], axis=AX.X, op=OP.max)
            cmx_ps = epps.tile([1, 28], FP32, tag="cmxp", name="cmxp")
            nc.tensor.transpose(cmx_ps[:], colmax[:], identity[0:28, 0:28])
            score_sb = ep.tile([1, 1], FP32, tag="sc", name="sc")
            nc.vector.tensor_reduce(score_sb[:], cmx_ps[:], axis=AX.X, op=OP.max)
            nc.sync.dma_start(score_d[:], score_sb[:])

            # ---- raw map: raw = R @ ps @ C via two matmul stages ----
            d_ps = epps.tile([28, 224], FP32, tag="dps", name="dps")
            nc.tensor.matmul(d_ps[:], ps28T[:], cm_sb[:])  # [28(h), 224(wo)]
            d_sb = ep.tile([28, 224], FP32, tag="dsb", name="dsb")
            nc.scalar.copy(d_sb[:], d_ps[:])
            for m in range(2):
                raw_ps = epps.tile([112, 224], FP32, tag="rawp", name="rawp")
                nc.tensor.matmul(raw_ps[:], rm_sb[:, 112 * m : 112 * (m + 1)], d_sb[:])
                raw_sb = ep.tile([112, 224], FP32, tag="raws", name="raws")
                nc.scalar.copy(raw_sb[:], raw_ps[:])
                nc.sync.dma_start(raw_d[112 * m : 112 * (m + 1), :], raw_sb[:])

    nc.finalize()
    return nc


_NC_CACHE = {}


def _get_nc():
    if "nc" not in _NC_CACHE:
        _NC_CACHE["nc"] = _build_nc()
    return _NC_CACHE["nc"]


def _interp_matrix(L, O):
    """W s.t. (W @ v) = torch bilinear resize (align_corners=False) of v: [O, L]."""
    W = np.zeros((O, L), np.float32)
    src = (np.arange(O, dtype=np.float64) + 0.5) * (L / O) - 0.5
    src = np.clip(src, 0.0, L - 1)
    i0 = np.floor(src).astype(np.int64)
    i1 = np.minimum(i0 + 1, L - 1)
    f = (src - i0).astype(np.float32)
    W[np.arange(O), i0] += 1.0 - f
    W[np.arange(O), i1] += f
    return W


def _stream(x):
    """[C,H,W] -> scrambled patch stream in (kj, w, c, ki, h) order (pure gather)."""
    A = np.pad(x, ((0, 0), (1, 1), (1, 1)))
    SW = sliding_window_view(A, (3, 3), axis=(1, 2))  # [C,H,W,ki,kj]
    return SW.transpose(4, 2, 0, 3, 1).reshape(-1)


def _prep_in_maps(feat1, feat2, memory):
    # memory prep: bf16, pad 20000 -> 20096 with copies of row 0 (min-idempotent)
    mem_bf = memory.astype(ml_dtypes.bfloat16)
    mem_pad = np.concatenate(
        [mem_bf, np.broadcast_to(mem_bf[0], (NCHUNK * P - memory.shape[0], 1024))], 0
    )
    # memt[c, p, ki, m] = mem_pad[c*128+m, ki*128+p]
    memt = np.ascontiguousarray(
        mem_pad.reshape(NCHUNK, P, 8, P).transpose(0, 3, 2, 1)
    )
    mmn = (mem_pad.astype(np.float64) ** 2).sum(1)
    mmn_t = np.ascontiguousarray(mmn.reshape(NCHUNK, P).T).astype(np.float32)
    wT = np.ascontiguousarray(_interp_matrix(28, 224).T)  # [28, 224]

    feat1 = feat1.astype(np.float32)
    feat2 = feat2.astype(np.float32)
    s1 = [
        np.ascontiguousarray(
            _stream(feat1[b]).reshape(NQ, 512, 9).transpose(1, 2, 0)
        )
        .reshape(4, P, 9, NQ)
        .astype(ml_dtypes.bfloat16)
        for b in range(8)
    ]
    s2 = [
        np.ascontiguousarray(
            _stream(feat2[b]).reshape(196, 512, 18).transpose(1, 2, 0)
        )
        .reshape(4, P, 18, 196)
        .astype(ml_dtypes.bfloat16)
        for b in range(8)
    ]

    return [
        {
            "s1": s1[b],
            "s2": s2[b],
            "memt": memt,
            "mmn": mmn_t,
            "rmat": wT,
            "cmat": wT,
        }
        for b in range(8)
    ]


def kernel(feat1, feat2, memory):
    nc = _get_nc()
    in_maps = _prep_in_maps(feat1, feat2, memory)
    res = run_bass_kernel_spmd(nc, in_maps, core_ids=list(range(8)))

    image_scores = np.array(
        [res.results[b]["score"][0, 0] for b in range(8)], dtype=np.float32
    )
    raw_maps = np.stack([res.results[b]["raw"] for b in range(8)]).astype(np.float32)
    return image_scores, raw_maps


# revision 34
# speedup vs baseline: 1.0100x; 1.0100x over previous
"""PatchCore on 8 Trainium2 cores.

Pipeline per image (one image per core, memory bank replicated):
  - Host: materialize the reference's scrambled patch stream (pure gather, no
    arithmetic) in group-major layout; device computes all sums.
    feats[:, j<512]  = (sum of 9-group + center)/10 of the feat1 stream.
    feats[:, 512+j'] = bilinear14->28(18-group sums of the feat2 stream)/18.
  - kNN: d2 = ||q||^2 - 2 q.m + ||m||^2 over 20096 (padded) memory rows in bf16
    on the PE array, running min on DVE, -2s+mm fused on ACT.
  - sqrt + 28->224 bilinear resize as two interpolation matmuls.
"""

from contextlib import ExitStack

import ml_dtypes
import numpy as np
from numpy.lib.stride_tricks import sliding_window_view

import concourse.bacc as bacc
import concourse.bass as bass
import concourse.mybir as mybir
import concourse.tile as tile
from concourse.bass_utils import run_bass_kernel_spmd
from concourse.masks import make_identity

P = 128
NQ = 784          # 28*28 queries per image
QB = 392          # query block (2 blocks per psum-bank row budget)
NCHUNK = 157      # 20096 padded memory rows / 128
FP32 = mybir.dt.float32
BF16 = mybir.dt.bfloat16
AF = mybir.ActivationFunctionType
OP = mybir.AluOpType
AX = mybir.AxisListType


def _upsample2x(nc, pool, x, H, W, tag):
    """Bilinear align_corners=False 2x upsample of last axis: [128,H,W]->[128,H,2W]."""
    u = pool.tile([P, H, W], FP32, tag=f"{tag}_u", name=f"{tag}_u")
    nc.vector.tensor_scalar_mul(u[:], x[:], 0.25)
    v = pool.tile([P, H, W], FP32, tag=f"{tag}_v", name=f"{tag}_v")
    nc.vector.tensor_sub(v[:], x[:], u[:])
    o = pool.tile([P, H, 2 * W], FP32, tag=f"{tag}_o", name=f"{tag}_o")
    W2 = 2 * W
    # out[0] = x[0]; out[2W-1] = x[W-1]
    nc.vector.tensor_copy(o[:, :, 0 :: W2 - 1], x[:, :, 0 :: W - 1])
    # out[2t] = 0.25 x[t-1] + 0.75 x[t], t=1..W-1
    nc.vector.tensor_add(o[:, :, 2 : W2 - 1 : 2], u[:, :, 0 : W - 1], v[:, :, 1:W])
    # out[2t+1] = 0.75 x[t] + 0.25 x[t+1], t=0..W-2
    nc.vector.tensor_add(o[:, :, 1 : W2 - 2 : 2], v[:, :, 0 : W - 1], u[:, :, 1:W])
    return o


def _build_nc():
    nc = bacc.Bacc("TRN2", target_bir_lowering=False)

    s1_d = nc.dram_tensor("s1", [4, P, 9, NQ], BF16, kind="ExternalInput")
    s2_d = nc.dram_tensor("s2", [4, P, 18, 196], BF16, kind="ExternalInput")
    mt_d = nc.dram_tensor("memt", [NCHUNK, P, 8, P], BF16, kind="ExternalInput")
    mmn_d = nc.dram_tensor("mmn", [P, NCHUNK], FP32, kind="ExternalInput")
    rm_d = nc.dram_tensor("rmat", [28, 224], FP32, kind="ExternalInput")
    cm_d = nc.dram_tensor("cmat", [28, 224], FP32, kind="ExternalInput")
    raw_d = nc.dram_tensor("raw", [224, 224], FP32, kind="ExternalOutput")
    score_d = nc.dram_tensor("score", [1, 1], FP32, kind="ExternalOutput")

    with tile.TileContext(nc) as tc, ExitStack() as ctx:
        # ---- persistent tiles ----
        def _tile(shape, dtype, name):
            t, f = tc.tile(shape, dtype, name=name)
            ctx.callback(f)
            return t

        identity = _tile([P, P], FP32, name="identity")
        make_identity(nc, identity)
        ones = _tile([P, 1], FP32, name="ones")
        nc.vector.memset(ones[:], 1.0)
        runmin = _tile([P, NQ], FP32, name="runmin")
        nc.vector.memset(runmin[:], 3.0e38)
        sq_acc = _tile([P, NQ], FP32, name="sq_acc")
        mmn_sb = _tile([P, NCHUNK], FP32, name="mmn_sb")
        nc.sync.dma_start(mmn_sb[:], mmn_d[:])
        rm_sb = _tile([28, 224], FP32, name="rm_sb")
        nc.sync.dma_start(rm_sb[:], rm_d[:])
        cm_sb = _tile([28, 224], FP32, name="cm_sb")
        nc.sync.dma_start(cm_sb[:], cm_d[:])
        ps28T = _tile([28, 28], FP32, name="ps28T")
        e_bf = [_tile([P, NQ], BF16, name=f"ebf{i}") for i in range(8)]

        emb = tc.alloc_tile_pool(name="emb", bufs=2)

        # ---- E1: feats cols 0..511 = (9-group sum + center)/10 ----
        # tree-reduce keeps reader-instruction count per DMA'd tile low
        # (hardware DMA has a small sync-wait budget).
        for i in range(4):
            x = emb.tile([P, 9, NQ], BF16, tag="s1in", name="s1in")
            nc.sync.dma_start(x[:], s1_d[i])
            t4 = emb.tile([P, 4, NQ], FP32, tag="e1t4", name="e1t4")
            nc.vector.tensor_add(t4[:], x[:, 0:4, :], x[:, 4:8, :])
            c2 = emb.tile([P, NQ], FP32, tag="e1c2", name="e1c2")
            nc.vector.tensor_add(c2[:], x[:, 4, :], x[:, 8, :])
            t2 = emb.tile([P, 2, NQ], FP32, tag="e1t2", name="e1t2")
            nc.vector.tensor_add(t2[:], t4[:, 0:2, :], t4[:, 2:4, :])
            acc = emb.tile([P, NQ], FP32, tag="e1acc", name="e1acc")
            nc.vector.tensor_add(acc[:], t2[:, 0, :], t2[:, 1, :])
            nc.vector.tensor_add(acc[:], acc[:], c2[:])
            nc.scalar.activation(e_bf[i][:], acc[:], AF.Copy, scale=0.1)

        # ---- E2: cols 512..1023 = bilinear14->28(18-group sums)/18 ----
        for i in range(4):
            y = emb.tile([P, 18, 196], BF16, tag="s2in", name="s2in")
            nc.sync.dma_start(y[:], s2_d[i])
            r9 = emb.tile([P, 9, 196], FP32, tag="e2r9", name="e2r9")
            nc.vector.tensor_add(r9[:], y[:, 0:9, :], y[:, 9:18, :])
            r4 = emb.tile([P, 4, 196], FP32, tag="e2r4", name="e2r4")
            nc.vector.tensor_add(r4[:], r9[:, 0:4, :], r9[:, 4:8, :])
            r2 = emb.tile([P, 2, 196], FP32, tag="e2r2", name="e2r2")
            nc.vector.tensor_add(r2[:], r4[:, 0:2, :], r4[:, 2:4, :])
            w2 = emb.tile([P, 14, 14], FP32, tag="w2", name="w2")
            w2f = w2.rearrange("p h w -> p (h w)")
            nc.vector.tensor_add(w2f, r2[:, 0, :], r2[:, 1, :])
            nc.vector.tensor_add(w2f, w2f, r9[:, 8, :])
            ow = _upsample2x(nc, emb, w2, 14, 14, "w")  # [128,14,28]
            ohin = ow.transpose([0, 2, 1])  # view [128,28,14]: expand h axis next
            oh = _upsample2x(nc, emb, ohin, 28, 14, "h")  # [128,28,28] as (w,h)
            # oh free layout is (w, h); rearrange view back to (h, w) row-major q
            nc.scalar.activation(
                e_bf[4 + i].rearrange("p (h w) -> p h w", h=28),
                oh.transpose([0, 2, 1]),
                AF.Copy,
                scale=1.0 / 18.0,
            )

        # ---- qq partials: sq_acc[p,q] = sum_i e_bf[i][p,q]^2 ----
        for i in range(8):
            sqt = emb.tile([P, NQ], FP32, tag="sqt", name="sqt")
            nc.scalar.activation(sqt[:], e_bf[i][:], AF.Square)
            if i == 0:
                nc.vector.tensor_copy(sq_acc[:], sqt[:])
            else:
                nc.vector.tensor_add(sq_acc[:], sq_acc[:], sqt[:])
        emb.release()

        # ---- main kNN loop over 157 memory chunks ----
        mt_pool = ctx.enter_context(tc.tile_pool(name="mtp", bufs=4))
        tpool = ctx.enter_context(tc.tile_pool(name="tp", bufs=2))
        with tc.tile_pool(name="mmps", bufs=2, space=bass.MemorySpace.PSUM) as mmps:
            for c in range(NCHUNK):
                mt = mt_pool.tile([P, 8, P], BF16, tag="mt", name="mt")
                nc.sync.dma_start(mt[:], mt_d[c])
                for blk in range(2):
                    ps = mmps.tile([P, QB], FP32, tag=f"ps{blk}", name="ps")
                    for ki in range(8):
                        nc.tensor.matmul(
                            ps[:],
                            mt[:, ki, :],
                            e_bf[ki][:, blk * QB : (blk + 1) * QB],
                            start=(ki == 0),
                            stop=(ki == 7),
                        )
                    tt = tpool.tile([P, QB], FP32, tag=f"tt{blk}", name="tt")
                    nc.scalar.activation(
                        tt[:], ps[:], AF.Identity, bias=mmn_sb[:, c : c + 1], scale=-2.0
                    )
                    nc.vector.tensor_tensor(
                        runmin[:, blk * QB : (blk + 1) * QB],
                        runmin[:, blk * QB : (blk + 1) * QB],
                        tt[:],
                        op=OP.min,
                    )

        # ---- epilogue: per-query min over partitions, +qq, sqrt ----
        ep = ctx.enter_context(tc.tile_pool(name="ep", bufs=2))
        with tc.tile_pool(name="epps", bufs=1, space=bass.MemorySpace.PSUM) as epps:
            for t in range(28):
                tr = epps.tile([28, P], FP32, tag="tr", bufs=2, name="tr")
                nc.tensor.transpose(tr[:], runmin[:, 28 * t : 28 * (t + 1)], identity[:])
                qq_ps = epps.tile([28, 1], FP32, tag="qq", bufs=2, name="qq")
                nc.tensor.matmul(qq_ps[:], sq_acc[:, 28 * t : 28 * (t + 1)], ones[:])
                qq_sb = ep.tile([28, 1], FP32, tag="qqs", name="qqs")
                nc.scalar.copy(qq_sb[:], qq_ps[:])
                mn = ep.tile([28, 1], FP32, tag="mn", name="mn")
                nc.vector.tensor_reduce(mn[:], tr[:], axis=AX.X, op=OP.min)
                d2 = ep.tile([28, 1], FP32, tag="d2", name="d2")
                nc.vector.tensor_scalar(
                    d2[:], mn[:], qq_sb[:], 0.0, op0=OP.add, op1=OP.max
                )
                nc.scalar.activation(ps28T[:, t : t + 1], d2[:], AF.Sqrt)

            # ---- image score: max over all 784 ----
            colmax = ep.tile([28, 1], FP32, tag="cmx", name="cmx")
            nc.vector.tensor_reduce(colmax[:], ps28T[:], axis=AX.X, op=OP.max)
            cmx_ps = epps.tile([1, 28], FP32, tag="cmxp", name="cmxp")
            nc.tensor.transpose(cmx_ps[:], colmax[:], identity[0:28, 0:28])
            score_sb = ep.tile([1, 1], FP32, tag="sc", name="sc")
            nc.vector.tensor_reduce(score_sb[:], cmx_ps[:], axis=AX.X, op=OP.max)
            nc.sync.dma_start(score_d[:], score_sb[:])

            # ---- raw map: raw = R @ ps @ C via two matmul stages ----
            d_ps = epps.tile([28, 224], FP32, tag="dps", name="dps")
            nc.tensor.matmul(d_ps[:], ps28T[:], cm_sb[:])  # [28(h), 224(wo)]
            d_sb = ep.tile([28, 224], FP32, tag="dsb", name="dsb")
            nc.scalar.copy(d_sb[:], d_ps[:])
            for m in range(2):
                raw_ps = epps.tile([112, 224], FP32, tag="rawp", name="rawp")
                nc.tensor.matmul(raw_ps[:], rm_sb[:, 112 * m : 112 * (m + 1)], d_sb[:])
                raw_sb = ep.tile([112, 224], FP32, tag="raws", name="raws")
                nc.scalar.copy(raw_sb[:], raw_ps[:])
                nc.sync.dma_start(raw_d[112 * m : 112 * (m + 1), :], raw_sb[:])

    nc.finalize()
    return nc


_NC_CACHE = {}


def _get_nc():
    if "nc" not in _NC_CACHE:
        _NC_CACHE["nc"] = _build_nc()
    return _NC_CACHE["nc"]


def _interp_matrix(L, O):
    """W s.t. (W @ v) = torch bilinear resize (align_corners=False) of v: [O, L]."""
    W = np.zeros((O, L), np.float32)
    src = (np.arange(O, dtype=np.float64) + 0.5) * (L / O) - 0.5
    src = np.clip(src, 0.0, L - 1)
    i0 = np.floor(src).astype(np.int64)
    i1 = np.minimum(i0 + 1, L - 1)
    f = (src - i0).astype(np.float32)
    W[np.arange(O), i0] += 1.0 - f
    W[np.arange(O), i1] += f
    return W


def _stream(x):
    """[C,H,W] -> scrambled patch stream in (kj, w, c, ki, h) order (pure gather)."""
    A = np.pad(x, ((0, 0), (1, 1), (1, 1)))
    SW = sliding_window_view(A, (3, 3), axis=(1, 2))  # [C,H,W,ki,kj]
    return SW.transpose(4, 2, 0, 3, 1).reshape(-1)


def _prep_in_maps(feat1, feat2, memory):
    # memory prep: bf16, pad 20000 -> 20096 with copies of row 0 (min-idempotent)
    mem_bf = memory.astype(ml_dtypes.bfloat16)
    mem_pad = np.concatenate(
        [mem_bf, np.broadcast_to(mem_bf[0], (NCHUNK * P - memory.shape[0], 1024))], 0
    )
    # memt[c, p, ki, m] = mem_pad[c*128+m, ki*128+p]
    memt = np.ascontiguousarray(
        mem_pad.reshape(NCHUNK, P, 8, P).transpose(0, 3, 2, 1)
    )
    mmn = (mem_pad.astype(np.float64) ** 2).sum(1)
    mmn_t = np.ascontiguousarray(mmn.reshape(NCHUNK, P).T).astype(np.float32)
    wT = np.ascontiguousarray(_interp_matrix(28, 224).T)  # [28, 224]

    feat1 = feat1.astype(np.float32)
    feat2 = feat2.astype(np.float32)
    s1 = [
        np.ascontiguousarray(
            _stream(feat1[b]).reshape(NQ, 512, 9).transpose(1, 2, 0)
        )
        .reshape(4, P, 9, NQ)
        .astype(ml_dtypes.bfloat16)
        for b in range(8)
    ]
    s2 = [
        np.ascontiguousarray(
            _stream(feat2[b]).reshape(196, 512, 18).transpose(1, 2, 0)
        )
        .reshape(4, P, 18, 196)
        .astype(ml_dtypes.bfloat16)
        for b in range(8)
    ]

    return [
        {
            "s1": s1[b],
            "s2": s2[b],
            "memt": memt,
            "mmn": mmn_t,
            "rmat": wT,
            "cmat": wT,
        }
        for b in range(8)
    ]


def kernel(feat1, feat2, memory):
    nc = _get_nc()
    in_maps = _prep_in_maps(feat1, feat2, memory)
    res = run_bass_kernel_spmd(nc, in_maps, core_ids=list(range(8)))

    image_scores = np.array(
        [res.results[b]["score"][0, 0] for b in range(8)], dtype=np.float32
    )
    raw_maps = np.stack([res.results[b]["raw"] for b in range(8)]).astype(np.float32)
    return image_scores, raw_maps
